# revision 27
# baseline (speedup 1.0000x reference)
import sys

sys.path.insert(0, "/opt/trn_rl_repo")

import numpy as np

from concourse import bass, bacc, mybir
import concourse.tile as tile
from concourse.bass_utils import run_bass_kernel_spmd

# Problem constants (hardcoded; kernel.py must be self-contained).
N = 200000
C_IN = 64
C_OUT = 16
K3 = 27
KW = K3 * C_OUT  # 432
OUT_SP = (998, 998, 38)
SENT = OUT_SP[0] * OUT_SP[1] * OUT_SP[2]  # 37848152
NK = N * K3

N_CORES = 8
PER = N // N_CORES            # 25000 points per core
PAD = 25088                   # 196 * 128
NCHUNK = PAD // 128           # 196
G = 7                         # chunks per output group
NGRP = NCHUNK // G            # 28

# Graded input tiles: tiny first tile so the first matmul's DMA dependency
# is small; all fetches are issued upfront (DMA is 8x faster than the
# tensor engine consumes, and front-loading moves input queue time into
# the period before the output stream saturates the queues).
TILE_CHUNKS = [1, 2, 4, 7] + [14] * 13  # sums to 196

TRACE = False
LAST_RESULTS = None

_NC = None


def _build_nc():
    chunk_tile = []
    for t, nch in enumerate(TILE_CHUNKS):
        for k in range(nch):
            chunk_tile.append((t, k))
    assert len(chunk_tile) == NCHUNK

    nc = bacc.Bacc("TRN2", target_bir_lowering=False, debug=False)
    featsT = nc.dram_tensor("featsT", [C_IN, PAD], mybir.dt.float16, kind="ExternalInput")
    w_all = nc.dram_tensor("w_all", [C_IN, KW], mybir.dt.float16, kind="ExternalInput")
    # y[g, p, j*432+o] = contribution row n = (g*7+j)*128 + p
    y = nc.dram_tensor("y", [NGRP, 128, G * KW], mybir.dt.float16, kind="ExternalOutput")

    with tile.TileContext(nc) as tc:
        with (
            tc.tile_pool(name="const", bufs=1) as constp,
            tc.tile_pool(name="inp", bufs=1) as inp,
            tc.tile_pool(name="outp", bufs=8) as outp,
            tc.tile_pool(name="psum", bufs=8, space="PSUM") as psump,
        ):
            w_sb = constp.tile([C_IN, KW], mybir.dt.float16)

            ftiles = [
                inp.tile([C_IN, TILE_CHUNKS[t] * 128], mybir.dt.float16, name=f"ftile{t}")
                for t in range(len(TILE_CHUNKS))
            ]

            col = 0
            starts = []
            for t, nch in enumerate(TILE_CHUNKS):
                starts.append(col)
                col += nch * 128

            # First MM depends only on ftile0 (16KB) + w (55KB).
            nc.sync.dma_start(ftiles[0][:], featsT[:, 0:TILE_CHUNKS[0] * 128])
            nc.sync.dma_start(w_sb[:], w_all[:])
            for t in range(1, len(TILE_CHUNKS)):
                nc.sync.dma_start(
                    ftiles[t][:], featsT[:, starts[t]:starts[t] + TILE_CHUNKS[t] * 128]
                )

            copy_engines = ["v", "s"]

            for g in range(NGRP):
                out_sb = outp.tile([128, G * KW], mybir.dt.float16)
                for j in range(G):
                    c = g * G + j
                    t, k = chunk_tile[c]
                    ps = psump.tile([128, KW], mybir.dt.float32)
                    nc.tensor.matmul(
                        ps[:],
                        lhsT=ftiles[t][:, k * 128:(k + 1) * 128],
                        rhs=w_sb[:],
                        start=True,
                        stop=True,
                    )
                    dst = out_sb[:, j * KW:(j + 1) * KW]
                    if copy_engines[(g * G + j) % 2] == "v":
                        nc.vector.tensor_copy(dst, ps[:])
                    else:
                        nc.scalar.activation(
                            dst, ps[:], mybir.ActivationFunctionType.Copy
                        )
                    if g == NGRP - 1:
                        # Split the final group's out DMA per chunk so the
                        # tail drains as soon as each copy lands.
                        nc.sync.dma_start(
                            y[g, :, j * KW:(j + 1) * KW], dst
                        )
                if g < NGRP - 1:
                    nc.sync.dma_start(y[g], out_sb[:])
    nc.compile()
    return nc


def _get_nc():
    global _NC
    if _NC is None:
        _NC = _build_nc()
    return _NC


def kernel(input, coords, W, bias):
    global LAST_RESULTS
    feats = np.asarray(input, dtype=np.float32).astype(np.float16)
    # W_all[c, k*16+o] = W[k, c, o]
    w_all = np.ascontiguousarray(
        np.transpose(np.asarray(W, np.float32), (1, 0, 2)).reshape(C_IN, KW)
    ).astype(np.float16)

    in_maps = []
    for i in range(N_CORES):
        sh = feats[i * PER:(i + 1) * PER]  # [25000, 64]
        fT = np.zeros((C_IN, PAD), np.float16)
        fT[:, :PER] = sh.T
        in_maps.append({"featsT": fT, "w_all": w_all})

    nc = _get_nc()
    res = run_bass_kernel_spmd(nc, in_maps, list(range(N_CORES)), trace=TRACE)
    LAST_RESULTS = res

    # Reassemble per-core Y: [28, 128, 7*432] -> [25088, 432] -> [25000, 432]
    parts = []
    for i in range(N_CORES):
        yc = np.asarray(res.results[i]["y"]).astype(np.float32).reshape(NGRP, 128, G, KW)
        yc = yc.transpose(0, 2, 1, 3).reshape(PAD, KW)[:PER]
        parts.append(yc)
    Y = np.concatenate(parts, axis=0)          # [N, 432]
    C = Y.reshape(NK, C_OUT)                   # row n*27+k

    # Host rulebook: output coords per (point, tap)
    xyz = coords[:, 1:4].astype(np.int32)
    r = np.arange(3, dtype=np.int32)
    off = np.stack(np.meshgrid(r, r, r, indexing="ij"), axis=-1).reshape(K3, 3)
    oc = xyz[:, None, :] - off[None, :, :]     # [N, 27, 3]
    sp = np.array(OUT_SP, dtype=np.int32)
    valid = np.all((oc >= 0) & (oc < sp), axis=-1)  # [N, 27]
    lin = (
        oc[..., 0] * (OUT_SP[1] * OUT_SP[2])
        + oc[..., 1] * OUT_SP[2]
        + oc[..., 2]
    )
    lin = np.where(valid, lin, SENT).reshape(-1)    # [NK] int32

    order = np.argsort(lin, kind="stable")
    slin = lin[order]
    starts = np.flatnonzero(np.r_[True, slin[1:] != slin[:-1]])
    uniq_vals = slin[starts]
    U = len(starts)

    Csort = C[order]
    sums = np.add.reduceat(Csort, starts, axis=0)   # [U, 16]

    out = np.zeros((NK, C_OUT), np.float32)
    out[:U] = sums + bias[None, :].astype(np.float32)
    if uniq_vals[-1] == SENT:
        out[U - 1] = 0.0

    uniq = np.full(NK, SENT, np.int32)
    uniq[:U] = uniq_vals
    return out, uniq
